# revision 6
# baseline (speedup 1.0000x reference)
"""GumbelSelector Trainium2 kernel, v5: fp16-pair W1 + token-major epilogue.

vs v4: W1 ships as an fp16 PAIR (hi = fp16(W1), lo = fp16(W1 - hi); the lo
part is f16-subnormal, which the PE honors), and layer 1 accumulates all
four stationary parts into one PSUM group.  That removes W1's fp16
quantization from the z error entirely (~2.0e-4 of z-noise, worth ~5-6
dec flips of the 22-flip budget), leaving only the irreducible fp16
rounding of s.

Math: h = relu(s @ W1 + b1); z = h @ (W2[:,1]-W2[:,0])
  dec  = (z > -(b2[1]-b2[0])) (+ per-row LB correction)
  prob = sigmoid(z + (b2[1]-b2[0]))

vs v2: layer 2 runs with h-chunks as the matmul STATIONARY and w2d as the
moving operand, so z lands token-major in PSUM.  dec then comes from z
directly (no sigmoid-table error), the sigmoid runs on wide tiles instead
of [1,1024] strips, and dec/prob DMA out as dense blocks the host
unpermutes.  The fp32r ISA processes moving columns in PAIRS (min width
2), so w2d is shipped duplicated ([128,2]) and every z value lands twice;
the host reads every other column.  fp32r tiles are kept 8-byte-aligned
in SBUF (the fp32r weight fetch mispairs half-words otherwise).

Sharding: data-parallel over batch B=64 -> 8 cores x 8 rows.
"""

import sys

if "/opt/trn_rl_repo" not in sys.path:
    sys.path.insert(0, "/opt/trn_rl_repo")

import numpy as np

import concourse.bass as bass
import concourse.bass_isa as bass_isa
import concourse.mybir as mybir
import concourse.tile as tile
from concourse import bacc
from concourse.bass_utils import run_bass_kernel_spmd

B, N, D = 64, 4096, 256
HID = D // 2  # 128
NCORES = 8
BPC = B // NCORES          # batch rows per core
TOK = BPC * N              # 32768 tokens per core
SLAB = 2048                # tokens per DMA slab (one contiguous 1 MiB load)
NSLAB = TOK // SLAB
TS = 1024                  # tokens per compute tile (2 PSUM banks)
NTS = TOK // TS            # 32
NQ = TOK // 128            # token chunks per rep (256)
CPB = N // 128             # z chunks per b_row (32)
F32 = mybir.dt.float32
F32R = mybir.dt.float32r
F16 = mybir.dt.float16

_NC = None


def _build_nc(reps=1, variant="full"):
    nc = bacc.Bacc("TRN2", target_bir_lowering=False, debug=False)
    sS = nc.dram_tensor("sS", [NSLAB * 128, 2 * SLAB], F16, kind="ExternalInput")
    rnT = nc.dram_tensor("rnT", [128, 2 * NQ], F32, kind="ExternalInput")
    w1 = nc.dram_tensor("w1", [2 * D, HID], F16, kind="ExternalInput")
    b1 = nc.dram_tensor("b1", [HID, 1], F32, kind="ExternalInput")
    w2d = nc.dram_tensor("w2d", [HID, 2], F32R, kind="ExternalInput")
    b2d = nc.dram_tensor("b2d", [128, 1], F32, kind="ExternalInput")
    nb2d = nc.dram_tensor("nb2d", [128, 1], F32, kind="ExternalInput")
    decT = nc.dram_tensor("decT", [128, 2 * NQ], F16, kind="ExternalOutput")
    probT = nc.dram_tensor("probT", [128, 2 * NQ], F16, kind="ExternalOutput")

    AF = mybir.ActivationFunctionType
    ALU = mybir.AluOpType

    with tile.TileContext(nc) as tc:
        with (
            tc.tile_pool(name="consts", bufs=1) as consts,
            tc.tile_pool(name="iop", bufs=1) as iop,
            tc.tile_pool(name="sload", bufs=4) as sload,
            tc.tile_pool(name="hpool", bufs=4) as hpool,
            tc.tile_pool(name="otile", bufs=2) as otile,
            tc.tile_pool(name="spool", bufs=2) as spool,
            tc.tile_pool(name="phpool", bufs=2, space=bass.MemorySpace.PSUM) as phpool,
            tc.tile_pool(name="pzpool", bufs=2, space=bass.MemorySpace.PSUM) as pzpool,
        ):
            # fp32r tiles at 8-byte-aligned pool offsets (w2s before b1s)
            w1a = consts.tile([128, HID], F16)
            nc.sync.dma_start(w1a[:], w1[0:128, :])
            w1b = consts.tile([128, HID], F16)
            nc.sync.dma_start(w1b[:], w1[128:256, :])
            w1al = consts.tile([128, HID], F16)
            nc.sync.dma_start(w1al[:], w1[256:384, :])
            w1bl = consts.tile([128, HID], F16)
            nc.sync.dma_start(w1bl[:], w1[384:512, :])
            w2s = consts.tile([HID, 2], F32R)
            nc.sync.dma_start(w2s[:], w2d[:])
            b1s = consts.tile([HID, 1], F32)
            nc.sync.dma_start(b1s[:], b1[:])
            b2s = consts.tile([128, 1], F32)
            nc.sync.dma_start(b2s[:], b2d[:])
            nb2s = consts.tile([128, 1], F32)
            nc.sync.dma_start(nb2s[:], nb2d[:])
            rns = iop.tile([128, 2 * NQ], F32)
            nc.sync.dma_start(rns[:], rnT[:])

            for rep in range(reps):
                if variant == "dmaonly":
                    acc = iop.tile([128, 1], F16, tag="acc")
                    for si in range(NSLAB):
                        st = sload.tile([128, 2 * SLAB], F16, tag="st")
                        nc.sync.dma_start(st[:], sS[si * 128 : (si + 1) * 128, :])
                        nc.vector.tensor_reduce(
                            acc[:], st[:, 0:64], mybir.AxisListType.X, ALU.max
                        )
                    nc.sync.dma_start(decT[:, 0:1], acc[:])
                    nc.sync.dma_start(probT[:, 0:1], acc[:])
                    continue

                dec_t = otile.tile([128, 2 * NQ], F16, tag="dec")
                prob_t = otile.tile([128, 2 * NQ], F16, tag="prob")
                zt = pzpool.tile([128, 2 * NQ], F32, tag="zt")

                def epilogue(b):
                    # row b: dec/prob from its 32 (duplicated) z columns,
                    # plus the LB correction; then stream its outputs out
                    sl = slice(2 * CPB * b, 2 * CPB * (b + 1))
                    nc.vector.tensor_scalar(dec_t[:, sl], zt[:, sl],
                                            nb2s[:], None, ALU.is_gt)
                    nc.scalar.activation(prob_t[:, sl], zt[:, sl],
                                         AF.Sigmoid, bias=b2s[:])
                    dmx = spool.tile([128, 2], F32, tag="dmx")
                    nc.vector.tensor_reduce(dmx[:, 0:1], dec_t[:, sl],
                                            mybir.AxisListType.X, ALU.max)
                    nc.vector.tensor_reduce(dmx[:, 1:2], rns[:, sl],
                                            mybir.AxisListType.X, ALU.max)
                    ar = spool.tile([128, 2], F32, tag="ar")
                    nc.gpsimd.partition_all_reduce(ar[:], dmx[:], 128,
                                                   bass_isa.ReduceOp.max)
                    need = spool.tile([128, 1], F32, tag="need")
                    nc.vector.tensor_scalar(need[:], ar[:, 0:1], 0.0, None,
                                            ALU.is_equal)
                    fx = spool.tile([128, 2 * CPB], F16, tag="fx")
                    nc.vector.tensor_scalar(fx[:], rns[:, sl],
                                            ar[:, 1:2], need[:],
                                            ALU.is_equal, ALU.mult)
                    nc.vector.tensor_max(dec_t[:, sl], dec_t[:, sl], fx[:])
                    nc.sync.dma_start(decT[:, sl], dec_t[:, sl])
                    nc.sync.dma_start(probT[:, sl], prob_t[:, sl])

                st = None
                prev = None  # (h tile, ts index) pending layer-2
                for j in range(NTS):
                    toff = j * TS
                    si, hoff = toff // SLAB, toff % SLAB
                    if hoff == 0:
                        st = sload.tile([128, 2 * SLAB], F16, tag="st")
                        nc.sync.dma_start(st[:], sS[si * 128 : (si + 1) * 128, :])
                    ph = phpool.tile([128, TS], F32)
                    nc.tensor.matmul(ph[:, 0:512], w1a[:],
                                     st[:, hoff : hoff + 512],
                                     start=True, stop=False)
                    nc.tensor.matmul(ph[:, 512:1024], w1a[:],
                                     st[:, hoff + 512 : hoff + 1024],
                                     start=True, stop=False)
                    nc.tensor.matmul(ph[:, 0:512], w1b[:],
                                     st[:, SLAB + hoff : SLAB + hoff + 512],
                                     start=False, stop=False)
                    nc.tensor.matmul(ph[:, 512:1024], w1b[:],
                                     st[:, SLAB + hoff + 512 : SLAB + hoff + 1024],
                                     start=False, stop=False)
                    nc.tensor.matmul(ph[:, 0:512], w1al[:],
                                     st[:, hoff : hoff + 512],
                                     start=False, stop=False)
                    nc.tensor.matmul(ph[:, 512:1024], w1al[:],
                                     st[:, hoff + 512 : hoff + 1024],
                                     start=False, stop=False)
                    nc.tensor.matmul(ph[:, 0:512], w1bl[:],
                                     st[:, SLAB + hoff : SLAB + hoff + 512],
                                     start=False, stop=True)
                    nc.tensor.matmul(ph[:, 512:1024], w1bl[:],
                                     st[:, SLAB + hoff + 512 : SLAB + hoff + 1024],
                                     start=False, stop=True)
                    if variant == "mmonly":
                        continue
                    # relu split across DVE and ACT so neither engine
                    # carries the whole 34us stream pass
                    h = hpool.tile([128, TS], F32R)
                    if j % 2 == 0:
                        nc.vector.tensor_scalar(h[:], ph[:], b1s[:], 0.0,
                                                ALU.add, ALU.max)
                    else:
                        nc.scalar.activation(h[:], ph[:], AF.Relu,
                                             bias=b1s[:])
                    if variant == "mmrelu":
                        continue
                    # layer 2 of the PREVIOUS block: overlaps this block's
                    # relu so the PE never waits on the DVE
                    if prev is not None:
                        hp, jp = prev
                        for k in range(8):
                            q = 8 * jp + k
                            nc.tensor.matmul(
                                zt[:, 2 * q : 2 * q + 2],
                                hp[:, 128 * k : 128 * (k + 1)],
                                w2s[:], start=True, stop=True)
                        if jp % 4 == 3:
                            epilogue(jp // 4)
                    prev = (h, j)
                if variant in ("mmonly", "mmrelu"):
                    continue
                hp, jp = prev
                for k in range(8):
                    q = 8 * jp + k
                    nc.tensor.matmul(zt[:, 2 * q : 2 * q + 2],
                                     hp[:, 128 * k : 128 * (k + 1)],
                                     w2s[:], start=True, stop=True)
                epilogue(BPC - 1)



    nc.compile()
    return nc


def _get_nc():
    global _NC
    if _NC is None:
        _NC = _build_nc()
    return _NC


def _make_in_maps(s, W1, b1, W2, b2, rnoise):
    s16 = np.asarray(s, dtype=np.float16)
    w1hi = np.asarray(W1, dtype=np.float16)
    w1lo = (np.asarray(W1, dtype=np.float64)
            - w1hi.astype(np.float64)).astype(np.float16)
    w1 = np.ascontiguousarray(np.concatenate([w1hi, w1lo], axis=0))
    b1c = np.ascontiguousarray(b1, dtype=np.float32).reshape(HID, 1)
    w2dv = np.ascontiguousarray(
        np.asarray(W2[:, 1], np.float32) - np.asarray(W2[:, 0], np.float32)
    ).reshape(HID, 1)
    w2dc = np.repeat(w2dv, 2, axis=1)
    b2dv = np.float32(b2[1]) - np.float32(b2[0])
    b2dc = np.full((128, 1), b2dv, dtype=np.float32)
    nb2dc = np.full((128, 1), -b2dv, dtype=np.float32)
    rn = np.ascontiguousarray(rnoise, dtype=np.float32)

    sS = np.ascontiguousarray(
        s16.reshape(NCORES, NSLAB, SLAB, 2, 128).transpose(0, 1, 4, 3, 2)
    ).reshape(NCORES, NSLAB * 128, 2 * SLAB)
    # token-major rnoise, column-duplicated to match the paired z layout:
    # rnT[c][p, 2q+r] = rn_flat[c][q*128 + p]
    rnT1 = rn.reshape(NCORES, NQ, 128).transpose(0, 2, 1)
    rnT = np.ascontiguousarray(np.repeat(rnT1, 2, axis=2))
    return [
        {
            "sS": sS[c],
            "rnT": rnT[c],
            "w1": w1,
            "b1": b1c,
            "w2d": w2dc,
            "b2d": b2dc,
            "nb2d": nb2dc,
        }
        for c in range(NCORES)
    ]


def _assemble(results):
    # outputs are duplicated token-major [128, 2*NQ]:
    # flat[q*128 + p] = out[p, 2q]
    dec = np.concatenate(
        [r["decT"][:, 0::2].T.reshape(BPC, N) for r in results], axis=0
    ).astype(np.float32)
    prob = np.concatenate(
        [r["probT"][:, 0::2].T.reshape(BPC, N) for r in results], axis=0
    ).astype(np.float32)
    return dec, prob


def run(s, W1, b1, W2, b2, rnoise, trace=False):
    nc = _get_nc()
    in_maps = _make_in_maps(s, W1, b1, W2, b2, rnoise)
    res = run_bass_kernel_spmd(nc, in_maps, list(range(NCORES)), trace=trace)
    return _assemble(res.results), res


def kernel(s, W1, b1, W2, b2, rnoise):
    (dec, prob), _ = run(s, W1, b1, W2, b2, rnoise)
    return dec, prob
